# revision 38
# baseline (speedup 1.0000x reference)
"""GCN (2-layer + MLP head) on 8 NeuronCores — v4: dma_gather token fetch +
matmul aggregation, pipelined sub-AllGather exchange.

Per core (nodes dst-sharded, 12500 real / 12800 padded):
  GEMM: hw = (x @ W1) * dinv (node-major fp32, SBUF-resident self table);
  slice rows cast bf16 into 4 sub-slices [3200, 128] (cols 64:128 junk
  pad to honor dma_gather's 256 B min element). Each sub-AllGather fires
  as soon as its 25 producer tiles are written -> 4 sub-tables
  [25600, 128] bf16 in DRAM (int16-addressable).
  Edge tokens ordered (sub-table q, dst-block b); per (q,b) cell padded
  to a cross-core-common multiple of 128. gpsimd.dma_gather fetches
  1024 tokens/call (HW crashes above 1024 idxs/call; int16 idx wrapped
  [16, n/16], replicated across Q7 cores; token i -> out[i%128, i//128]).
  The whole idx table stays SBUF-resident (~57 KB/partition).
  Aggregation: per token-tile, M[t,d] = (dstrel[t]==d) built on DVE
  (is_equal vs iota, bf16), PE matmul M^T @ g accumulates a (q,b)
  segment in PSUM; folds into SBUF acc (copy q==0, add after).
  Pointwise per block is interleaved into the last quarter's folds so
  layer-2 sub-AllGathers (and layer-2 gathers) overlap layer-1's tail:
  h = relu((acc + hw_self)*dinv + b); layer-2 GEMM via PE transpose;
  head MLP per tile; out [2, 12800] per core.
  Gathers are issued round-robin over 4 SWDGE queues (num_swdge_queues=4)
  with deep g/mask pools, overlapping each call's ring/DMA tail with the
  next call's descriptor generation: the 864-call stream runs at ~4 us
  per 1024-token call and is the critical path; PE (~3.3 ms busy) and
  DVE (~2.8 ms) hide just under it.
"""
import numpy as np

import concourse.bacc as bacc
import concourse.mybir as mybir
from concourse import bass
from concourse.tile import TileContext
from concourse.bass_utils import run_bass_kernel_spmd
from concourse.masks import make_identity

N = 100000
NS_RAW = 12500
NS = 12800
NTILE = NS // 128          # 100
NBLK = 98                  # blocks containing real dsts
NQ = 4                     # table quarters (int16 index range)
SUB = NS // NQ             # 3200 slice rows per sub-AllGather
QROWS = 8 * SUB            # 25600 rows per sub-table
IN_CH, HID, HID2, OUT = 256, 64, 32, 2
CALL_T = 4                 # token-tiles per dma_gather call (1024 tokens;
                           # single_packet=True crashes above 1024 idxs, and
                           # single_packet=False costs 2.3x more per token)
PAD_SENT = -1000.0

_compiled = {}


def _build_schedule(src, dst):
    """Token schedule, shape-equalized across cores.

    Returns (per_core, tq, perms):
      per_core[c] = (idx16 [128, TOT4*8] int16 quarter-local row indices,
                     drel [128, TOT4] f32 dst-local-in-block / PAD_SENT)
      tq[q][b] = tiles for (quarter q, block b) (common across cores)
    """
    core = dst // NS_RAW
    dstl = (dst % NS_RAW).astype(np.int64)
    src_core = src // NS_RAW
    src_local = (src % NS_RAW).astype(np.int64)

    # Per-core node permutation: sort own dsts by in-degree desc, deal
    # round-robin into the 98 blocks so per-block token counts are nearly
    # equal (minimizes cross-core common padding). perm[old_local] = new.
    perms = []
    for c in range(8):
        degc = np.bincount(dstl[core == c], minlength=NS_RAW)
        order = np.argsort(-degc, kind="stable")
        ranks = np.arange(NS_RAW)
        newpos = (ranks % NBLK) * 128 + ranks // NBLK
        perm = np.empty(NS_RAW, np.int64)
        perm[order] = newpos
        perms.append(perm)
    permsA = np.stack(perms)                       # [8, NS_RAW]

    dstp = permsA[core, dstl]                      # permuted dst-local
    srcp = permsA[src_core, src_local]             # permuted src slot
    blk = dstp // 128
    # sub-table k holds slice rows [k*SUB, (k+1)*SUB) of every core,
    # concatenated core-major by the k-th sub-AllGather.
    qq = srcp // SUB
    rloc = (src_core * SUB + srcp % SUB).astype(np.int16)
    drel_all = (dstp - 128 * blk).astype(np.float32)

    # group per core by (quarter, block)
    cells = {}
    ntile = np.zeros((8, NQ, NBLK), np.int64)
    for c in range(8):
        m = core == c
        key = qq[m] * NBLK + blk[m]
        o = np.argsort(key, kind="stable")
        kk, rr, dd = key[o], rloc[m][o], drel_all[m][o]
        bounds = np.searchsorted(kk, np.arange(NQ * NBLK + 1))
        cells[c] = (bounds, rr, dd)
        cnt = bounds[1:] - bounds[:-1]
        ntile[c] = np.ceil(cnt / 128).reshape(NQ, NBLK).astype(np.int64)

    tq = np.maximum(ntile.max(axis=0), 1)          # [NQ, NBLK]
    TOT4 = int(tq.sum())

    # cell-aligned gather calls; per call, the cross-core-common trailing
    # pad region gets idx=-1 (the dma_gather ucode stops at the last valid
    # index, skipping those slots entirely). reg = slots before that region.
    cell_calls = {}
    regs = []
    for q in range(NQ):
        for b in range(NBLK):
            T = int(tq[q, b])
            cl = []
            for t0 in range(0, T, CALL_T):
                nt = min(CALL_T, T - t0)
                s0, s1 = t0 * 128, (t0 + nt) * 128
                r = 1
                for c in range(8):
                    lo, hi = cells[c][0][q * NBLK + b],                         cells[c][0][q * NBLK + b + 1]
                    rc = int(hi - lo)
                    r = max(r, min(max(rc - s0, 0), s1 - s0))
                cl.append((s0, s1, r))
                regs.append(r)
            cell_calls[(q, b)] = cl

    per_core = []
    for c in range(8):
        bounds, rr, dd = cells[c]
        r_parts, d_parts = [], []
        for cell in range(NQ * NBLK):
            lo, hi = bounds[cell], bounds[cell + 1]
            n = hi - lo
            q, b = cell // NBLK, cell % NBLK
            cap = int(tq[q, b]) * 128
            ra = np.zeros(cap, np.int16)
            da = np.full(cap, PAD_SENT, np.float32)
            ra[:n] = rr[lo:hi]
            da[:n] = dd[lo:hi]
            for (s0, s1, r) in cell_calls[(q, b)]:
                ra[s0 + r:s1] = -1
            r_parts.append(ra)
            d_parts.append(da)
        ra = np.concatenate(r_parts)
        da = np.concatenate(d_parts)
        # token t -> drel (partition t%128, column t//128)
        drel = da.reshape(TOT4, 128).T.astype(np.float32)
        # token t -> idx16 (partition t%16, column t//16), replicated x8
        i16 = ra.reshape(TOT4 * 8, 16).T.astype(np.int16)
        idx16 = np.tile(i16, (8, 1))
        per_core.append((np.ascontiguousarray(idx16),
                         np.ascontiguousarray(drel)))
    return per_core, tq, regs, perms


def _build_program(tq, regs):
    nc = bacc.Bacc(None, target_bir_lowering=False, num_swdge_queues=4)
    dt = mybir.dt
    P = nc.declare_dram_parameter
    TOT4 = int(tq.sum())
    xT = P("xT", [IN_CH, NS], dt.float32, isOutput=False)
    w1p = P("w1p", [128, 128], dt.float32, isOutput=False)
    w2 = P("w2", [HID, HID], dt.float32, isOutput=False)
    wh1 = P("wh1", [HID, HID2], dt.float32, isOutput=False)
    wh2 = P("wh2", [HID2, OUT], dt.float32, isOutput=False)
    b1f = P("b1f", [128, HID], dt.float32, isOutput=False)
    b2f = P("b2f", [128, HID], dt.float32, isOutput=False)
    bh1 = P("bh1", [HID2, 1], dt.float32, isOutput=False)
    bh2 = P("bh2", [OUT, 1], dt.float32, isOutput=False)
    dinvP = P("dinvP", [128, NTILE], dt.float32, isOutput=False)
    idxD = P("idxD", [128, TOT4 * 8], dt.int16, isOutput=False)
    dstrelP = P("dstrelP", [128, TOT4], dt.float32, isOutput=False)
    iotaP = P("iotaP", [128, 1024], dt.float32, isOutput=False)
    outT = P("outT", [OUT, NS], dt.float32, isOutput=True)

    # table rows padded to 128 bf16 cols (= 256 B, dma_gather min elem);
    # cols 64:128 are never written nor read (junk travels, is ignored).
    # 4 sub-slices / sub-tables per layer so each sub-AllGather fires as
    # soon as its 25 producer tiles are written (pipelined exchange).
    slice_d = [[nc.dram_tensor(f"slice{l}_{k}", [SUB, 128], dt.bfloat16)
                for k in range(NQ)] for l in (1, 2)]
    table_d = [[nc.dram_tensor(f"table{l}_{k}", [QROWS, 128], dt.bfloat16)
                for k in range(NQ)] for l in (1, 2)]

    iseq = mybir.AluOpType.is_equal
    relu = mybir.ActivationFunctionType.Relu
    copyf = mybir.ActivationFunctionType.Copy

    # per-tile (quarter, block) schedule; quarter-major then block
    col_block, col_q = [], []
    for q in range(NQ):
        for b in range(NBLK):
            col_block += [b] * int(tq[q, b])
            col_q += [q] * int(tq[q, b])
    # segment boundaries: tile index ranges per (q, b)
    seg_start = {}
    seg_stop = {}
    pos = 0
    qoff = [0]
    for q in range(NQ):
        for b in range(NBLK):
            t = int(tq[q, b])
            seg_start[pos] = (q, b)
            seg_stop[pos + t - 1] = (q, b)
            pos += t
        qoff.append(pos)

    with TileContext(nc) as tc:
        with tc.tile_pool(name="const", bufs=1) as cp, \
             tc.tile_pool(name="acc", bufs=1) as ap_, \
             tc.tile_pool(name="gath", bufs=10) as gp, \
             tc.tile_pool(name="work", bufs=3) as wp, \
             tc.tile_pool(name="m8", bufs=10) as mp, \
             tc.tile_pool(name="pst", bufs=2, space="PSUM") as ptp, \
             tc.tile_pool(name="psa", bufs=4, space="PSUM") as pap, \
             tc.tile_pool(name="psg", bufs=2, space="PSUM") as pgp:
            w1sb = cp.tile([128, 128], dt.float32)
            nc.sync.dma_start(out=w1sb[:], in_=w1p[:])
            w2sb = cp.tile([HID, HID], dt.float32)
            nc.sync.dma_start(out=w2sb[:], in_=w2[:])
            wh1sb = cp.tile([HID, HID2], dt.float32)
            nc.sync.dma_start(out=wh1sb[:], in_=wh1[:])
            wh2sb = cp.tile([HID2, OUT], dt.float32)
            nc.sync.dma_start(out=wh2sb[:], in_=wh2[:])
            b1sb = cp.tile([128, HID], dt.float32)
            nc.sync.dma_start(out=b1sb[:], in_=b1f[:])
            b2sb = cp.tile([128, HID], dt.float32)
            nc.sync.dma_start(out=b2sb[:], in_=b2f[:])
            bh1sb = cp.tile([HID2, 1], dt.float32)
            nc.sync.dma_start(out=bh1sb[:], in_=bh1[:])
            bh2sb = cp.tile([OUT, 1], dt.float32)
            nc.sync.dma_start(out=bh2sb[:], in_=bh2[:])
            dsb = cp.tile([128, NTILE], dt.float32)
            nc.sync.dma_start(out=dsb[:], in_=dinvP[:])
            ident = cp.tile([128, 128], dt.float32)
            make_identity(nc, ident[:])
            iotaf = cp.tile([128, 1024], dt.float32)
            nc.sync.dma_start(out=iotaf[:], in_=iotaP[:])
            iotab = cp.tile([128, 1024], dt.bfloat16)
            nc.vector.tensor_copy(iotab[:], iotaf[:])
            drf = cp.tile([128, TOT4], dt.float32)
            nc.sync.dma_start(out=drf[:], in_=dstrelP[:])
            drb = cp.tile([128, TOT4], dt.bfloat16)
            nc.vector.tensor_copy(drb[:], drf[:])
            idxsb = cp.tile([128, TOT4 * 8], dt.int16)
            nc.sync.dma_start(out=idxsb[:], in_=idxD[:])

            acc = ap_.tile([128, NBLK * HID], dt.float32)
            hwself = ap_.tile([128, NTILE * HID], dt.float32)

            def allgather(layer, k):
                nc.gpsimd.collective_compute(
                    "AllGather", mybir.AluOpType.bypass,
                    replica_groups=[list(range(8))],
                    ins=[slice_d[layer][k][:]],
                    outs=[table_d[layer][k][:]])

            def gemm1():
                for m in range(NTILE):
                    mc = slice(m * 128, (m + 1) * 128)
                    xa = wp.tile([128, 128], dt.float32, tag="xa")
                    nc.sync.dma_start(out=xa[:], in_=xT[0:128, mc])
                    xb = wp.tile([128, 128], dt.float32, tag="xb")
                    nc.sync.dma_start(out=xb[:], in_=xT[128:256, mc])
                    ps = pgp.tile([128, HID], dt.float32, tag="ps")
                    nc.tensor.matmul(ps[:], xa[:], w1sb[:, 0:HID],
                                     start=True, stop=False)
                    nc.tensor.matmul(ps[:], xb[:], w1sb[:, HID:128],
                                     start=False, stop=True)
                    hsl = hwself[:, m * HID:(m + 1) * HID]
                    nc.vector.tensor_scalar_mul(hsl, ps[:], dsb[:, m:m + 1])
                    sb16 = wp.tile([128, HID], dt.bfloat16, tag="sb16")
                    nc.scalar.activation(sb16[:], hsl, copyf)
                    nc.sync.dma_start(
                        out=slice_d[0][m // 25][
                            (m % 25) * 128:(m % 25 + 1) * 128, 0:HID],
                        in_=sb16[:])
                    if m % 25 == 24:
                        allgather(0, m // 25)

            def prime_gpool():
                for _ in range(10):
                    gt = gp.tile([128, CALL_T * 128], dt.bfloat16, tag="g")
                    nc.gpsimd.memset(gt[:], 0.0)

            def aggregate(layer, on_block_done):
                cur = None
                ncall = 0
                ci = 0
                pos0 = 0
                for q in range(NQ):
                    for b_ in range(NBLK):
                        T = int(tq[q, b_])
                        for t0 in range(0, T, CALL_T):
                            nw = min(CALL_T, T - t0)
                            c0 = pos0 + t0
                            reg = regs[ci]
                            ci += 1
                            # tiles fully inside the skip region have
                            # all-PAD masks (exactly zero contribution):
                            # drop their gather slots and matmuls, and move
                            # the segment stop to the last processed tile.
                            nproc = min(nw, -(-reg // 128))
                            last_call = t0 + nw == T
                            g = gp.tile([128, CALL_T * 128], dt.bfloat16,
                                        tag="g")
                            nc.gpsimd.dma_gather(
                                out_ap=g[:, 0:nproc * 128].rearrange(
                                    "p (k o) -> p k o", o=128),
                                in_ap=table_d[layer][q][:],
                                idxs_ap=idxsb[:, c0 * 8:(c0 + nproc) * 8],
                                num_idxs=nproc * 128,
                                num_idxs_reg=reg,
                                elem_size=128,
                                queue_num=ncall % 4,
                            )
                            ncall += 1
                            for c in range(c0, c0 + nproc):
                                k = c - c0
                                if k % 8 == 0:
                                    m8 = mp.tile([128, 1024], dt.bfloat16,
                                                 tag="m8")
                                    nm = min(8, nproc - k)
                                    d3b = drb[:, c:c + nm] \
                                        .rearrange("p (k o) -> p k o", o=1) \
                                        .broadcast_to((128, nm, 128))
                                    nc.vector.tensor_tensor(
                                        m8[:, 0:nm * 128].rearrange(
                                            "p (k o) -> p k o", o=128),
                                        iotab[:, 0:nm * 128].rearrange(
                                            "p (k o) -> p k o", o=128),
                                        d3b, iseq)
                                b = col_block[c]
                                if c in seg_start:
                                    agg_t = pap.tile([128, HID],
                                                     dt.float32, tag="agg")
                                    cur = agg_t
                                stop = last_call and c == c0 + nproc - 1
                                nc.tensor.matmul(
                                    cur[:],
                                    m8[:, (k % 8) * 128:(k % 8 + 1) * 128],
                                    g[:, k * 128:k * 128 + HID],
                                    start=c in seg_start,
                                    stop=stop)
                                if stop:
                                    asl = acc[:, b * HID:(b + 1) * HID]
                                    if q == 0:
                                        nc.vector.tensor_copy(asl, cur[:])
                                    else:
                                        nc.vector.tensor_add(asl, asl,
                                                             cur[:])
                                    if q == NQ - 1:
                                        on_block_done(b)
                        pos0 += T

            def pointwise_tile(layer, m):
                mc = slice(m * 128, (m + 1) * 128)
                hsl = hwself[:, m * HID:(m + 1) * HID]
                s = wp.tile([128, HID], dt.float32, tag="s")
                if m < NBLK:
                    nc.vector.tensor_add(
                        s[:], acc[:, m * HID:(m + 1) * HID], hsl)
                else:
                    nc.vector.tensor_copy(s[:], hsl)
                nc.vector.tensor_scalar_mul(s[:], s[:], dsb[:, m:m + 1])
                nc.vector.tensor_add(s[:], s[:],
                                     b1sb[:] if layer == 0 else b2sb[:])
                h = wp.tile([128, HID], dt.float32, tag="h")
                nc.scalar.activation(h[:], s[:], relu)
                pt = ptp.tile([128, 128], dt.float32, tag="tp")
                nc.tensor.transpose(pt[0:HID, :], h[:], ident[:])
                ht = wp.tile([HID, 128], dt.float32, tag="ht")
                nc.scalar.activation(ht[:], pt[0:HID, :], copyf)
                if layer == 0:
                    ps2 = pgp.tile([128, HID], dt.float32, tag="ps")
                    nc.tensor.matmul(ps2[:], ht[:], w2sb[:],
                                     start=True, stop=True)
                    nc.vector.tensor_scalar_mul(hsl, ps2[:],
                                                dsb[:, m:m + 1])
                    sb16 = wp.tile([128, HID], dt.bfloat16, tag="sb16b")
                    nc.scalar.activation(sb16[:], hsl, copyf)
                    nc.sync.dma_start(
                        out=slice_d[1][m // 25][
                            (m % 25) * 128:(m % 25 + 1) * 128, 0:HID],
                        in_=sb16[:])
                    if m % 25 == 24:
                        allgather(1, m // 25)
                else:
                    pz = ptp.tile([128, 128], dt.float32, tag="tp")
                    nc.tensor.matmul(pz[0:HID2, :], wh1sb[:], ht[:],
                                     start=True, stop=True)
                    zb = wp.tile([HID2, 128], dt.float32, tag="zb")
                    nc.scalar.activation(zb[:], pz[0:HID2, :], relu,
                                         bias=bh1sb[:])
                    po = ptp.tile([128, 128], dt.float32, tag="tp")
                    nc.tensor.matmul(po[0:OUT, :], wh2sb[:], zb[:],
                                     start=True, stop=True)
                    ob = wp.tile([OUT, 128], dt.float32, tag="ob")
                    nc.vector.tensor_scalar_add(ob[:], po[0:OUT, :],
                                                bh2sb[:])
                    nc.sync.dma_start(out=outT[:, mc], in_=ob[:])

            def on_block_done(layer):
                # pointwise for block b right after its final fold; tiles
                # 98/99 (self-only) ride along with block 97 so the last
                # sub-slice / output completes without a separate pass.
                def f(b):
                    pointwise_tile(layer, b)
                    if b == NBLK - 1:
                        for m in range(NBLK, NTILE):
                            pointwise_tile(layer, m)
                return f

            prime_gpool()
            gemm1()
            aggregate(0, on_block_done(0))
            aggregate(1, on_block_done(1))

    nc.finalize()
    return nc


def kernel(x, edge_index, W1, b1, W2, b2, Wh1, bh1, Wh2, bh2, _trace=False):
    x = np.asarray(x, np.float32)
    src = np.asarray(edge_index[0], np.int64)
    dst = np.asarray(edge_index[1], np.int64)

    per_core, tq, regs, perms = _build_schedule(src, dst)
    sig = tuple(tq.ravel()) + tuple(regs)
    if sig not in _compiled:
        _compiled[sig] = _build_program(tq, regs)
    nc = _compiled[sig]

    deg = np.bincount(dst, minlength=N).astype(np.float64) + 1.0
    dinv = (1.0 / np.sqrt(deg)).astype(np.float32)

    W1 = np.asarray(W1, np.float32)
    w1p = np.concatenate([W1[:128], W1[128:]], axis=1)
    b1f = np.tile(np.asarray(b1, np.float32)[None, :], (128, 1))
    b2f = np.tile(np.asarray(b2, np.float32)[None, :], (128, 1))
    bh1c = np.asarray(bh1, np.float32)[:, None]
    bh2c = np.asarray(bh2, np.float32)[:, None]
    iota = np.tile(np.arange(128, dtype=np.float32)[None, :], (128, 8))

    in_maps = []
    for c in range(8):
        idx16, drel = per_core[c]
        xs = np.zeros((NS, IN_CH), np.float32)
        xs[perms[c]] = x[c * NS_RAW:(c + 1) * NS_RAW]
        dv = np.ones(NS, np.float32)
        dv[perms[c]] = dinv[c * NS_RAW:(c + 1) * NS_RAW]
        in_maps.append({
            "xT": np.ascontiguousarray(xs.T),
            "w1p": np.ascontiguousarray(w1p),
            "w2": np.asarray(W2, np.float32),
            "wh1": np.asarray(Wh1, np.float32),
            "wh2": np.asarray(Wh2, np.float32),
            "b1f": b1f, "b2f": b2f, "bh1": bh1c, "bh2": bh2c,
            "dinvP": np.ascontiguousarray(dv.reshape(NTILE, 128).T),
            "idxD": idx16,
            "dstrelP": drel,
            "iotaP": iota,
        })

    res = run_bass_kernel_spmd(nc, in_maps, list(range(8)), trace=_trace)
    out = np.empty((N, OUT), np.float32)
    for c in range(8):
        out[c * NS_RAW:(c + 1) * NS_RAW] = res.results[c]["outT"].T[perms[c]]
    if _trace:
        kernel.last_results = res
    return out


# revision 40
# speedup vs baseline: 1.0022x; 1.0022x over previous
"""GCN (2-layer + MLP head) on 8 NeuronCores — v4: dma_gather token fetch +
matmul aggregation, pipelined sub-AllGather exchange.

Per core (nodes dst-sharded, 12500 real / 12800 padded):
  GEMM: hw = (x @ W1) * dinv (node-major fp32, SBUF-resident self table);
  slice rows cast bf16 into 4 sub-slices [3200, 128] (cols 64:128 junk
  pad to honor dma_gather's 256 B min element). Each sub-AllGather fires
  as soon as its 25 producer tiles are written -> 4 sub-tables
  [25600, 128] bf16 in DRAM (int16-addressable).
  Edge tokens ordered (sub-table q, dst-block b); per (q,b) cell padded
  to a cross-core-common multiple of 128. gpsimd.dma_gather fetches
  1024 tokens/call (HW crashes above 1024 idxs/call; int16 idx wrapped
  [16, n/16], replicated across Q7 cores; token i -> out[i%128, i//128]).
  The whole idx table stays SBUF-resident (~57 KB/partition).
  Aggregation: per token-tile, M[t,d] = (dstrel[t]==d) built on DVE
  (is_equal vs iota, bf16), PE matmul M^T @ g accumulates a (q,b)
  segment in PSUM; folds into SBUF acc (copy q==0, add after).
  Pointwise per block is interleaved into the last quarter's folds so
  layer-2 sub-AllGathers (and layer-2 gathers) overlap layer-1's tail:
  h = relu((acc + hw_self)*dinv + b); layer-2 GEMM via PE transpose;
  head MLP per tile; out [2, 12800] per core.
  Gathers are issued round-robin over 4 SWDGE queues (num_swdge_queues=4)
  with deep g/mask pools, overlapping each call's ring/DMA tail with the
  next call's descriptor generation: the 864-call stream runs at ~4 us
  per 1024-token call and is the critical path; PE (~3.3 ms busy) and
  DVE (~2.8 ms) hide just under it.
"""
import numpy as np

import concourse.bacc as bacc
import concourse.mybir as mybir
from concourse import bass
from concourse.tile import TileContext
from concourse.bass_utils import run_bass_kernel_spmd
from concourse.masks import make_identity

N = 100000
NS_RAW = 12500
NS = 12800
NTILE = NS // 128          # 100
NBLK = 98                  # blocks containing real dsts
NQ = 4                     # table quarters (int16 index range)
SUB = NS // NQ             # 3200 slice rows per sub-AllGather
QROWS = 8 * SUB            # 25600 rows per sub-table
IN_CH, HID, HID2, OUT = 256, 64, 32, 2
CALL_T = 4                 # token-tiles per dma_gather call (1024 tokens;
                           # single_packet=True crashes above 1024 idxs, and
                           # single_packet=False costs 2.3x more per token)
PAD_SENT = -1000.0

_compiled = {}


def _build_schedule(src, dst):
    """Token schedule, shape-equalized across cores.

    Returns (per_core, tq, perms):
      per_core[c] = (idx16 [128, TOT4*8] int16 quarter-local row indices,
                     drel [128, TOT4] f32 dst-local-in-block / PAD_SENT)
      tq[q][b] = tiles for (quarter q, block b) (common across cores)
    """
    core = dst // NS_RAW
    dstl = (dst % NS_RAW).astype(np.int64)
    src_core = src // NS_RAW
    src_local = (src % NS_RAW).astype(np.int64)

    # Per-core node permutation: sort own dsts by in-degree desc, deal
    # round-robin into the 98 blocks so per-block token counts are nearly
    # equal (minimizes cross-core common padding). perm[old_local] = new.
    perms = []
    for c in range(8):
        degc = np.bincount(dstl[core == c], minlength=NS_RAW)
        order = np.argsort(-degc, kind="stable")
        ranks = np.arange(NS_RAW)
        newpos = (ranks % NBLK) * 128 + ranks // NBLK
        perm = np.empty(NS_RAW, np.int64)
        perm[order] = newpos
        perms.append(perm)
    permsA = np.stack(perms)                       # [8, NS_RAW]

    dstp = permsA[core, dstl]                      # permuted dst-local
    srcp = permsA[src_core, src_local]             # permuted src slot
    blk = dstp // 128
    # sub-table k holds slice rows [k*SUB, (k+1)*SUB) of every core,
    # concatenated core-major by the k-th sub-AllGather.
    qq = srcp // SUB
    rloc = (src_core * SUB + srcp % SUB).astype(np.int16)
    drel_all = (dstp - 128 * blk).astype(np.float32)

    # group per core by (quarter, block)
    cells = {}
    ntile = np.zeros((8, NQ, NBLK), np.int64)
    for c in range(8):
        m = core == c
        key = qq[m] * NBLK + blk[m]
        o = np.argsort(key, kind="stable")
        kk, rr, dd = key[o], rloc[m][o], drel_all[m][o]
        bounds = np.searchsorted(kk, np.arange(NQ * NBLK + 1))
        cells[c] = (bounds, rr, dd)
        cnt = bounds[1:] - bounds[:-1]
        ntile[c] = np.ceil(cnt / 128).reshape(NQ, NBLK).astype(np.int64)

    tq = np.maximum(ntile.max(axis=0), 1)          # [NQ, NBLK]
    TOT4 = int(tq.sum())

    # cell-aligned gather calls; per call, the cross-core-common trailing
    # pad region gets idx=-1 (the dma_gather ucode stops at the last valid
    # index, skipping those slots entirely). reg = slots before that region.
    cell_calls = {}
    regs = []
    for q in range(NQ):
        for b in range(NBLK):
            T = int(tq[q, b])
            cl = []
            for t0 in range(0, T, CALL_T):
                nt = min(CALL_T, T - t0)
                s0, s1 = t0 * 128, (t0 + nt) * 128
                r = 1
                for c in range(8):
                    lo, hi = cells[c][0][q * NBLK + b],                         cells[c][0][q * NBLK + b + 1]
                    rc = int(hi - lo)
                    r = max(r, min(max(rc - s0, 0), s1 - s0))
                cl.append((s0, s1, r))
                regs.append(r)
            cell_calls[(q, b)] = cl

    per_core = []
    for c in range(8):
        bounds, rr, dd = cells[c]
        r_parts, d_parts = [], []
        for cell in range(NQ * NBLK):
            lo, hi = bounds[cell], bounds[cell + 1]
            n = hi - lo
            q, b = cell // NBLK, cell % NBLK
            cap = int(tq[q, b]) * 128
            ra = np.zeros(cap, np.int16)
            da = np.full(cap, PAD_SENT, np.float32)
            ra[:n] = rr[lo:hi]
            da[:n] = dd[lo:hi]
            for (s0, s1, r) in cell_calls[(q, b)]:
                ra[s0 + r:s1] = -1
            r_parts.append(ra)
            d_parts.append(da)
        ra = np.concatenate(r_parts)
        da = np.concatenate(d_parts)
        # token t -> drel (partition t%128, column t//128)
        drel = da.reshape(TOT4, 128).T.astype(np.float32)
        # token t -> idx16 (partition t%16, column t//16), replicated x8
        i16 = ra.reshape(TOT4 * 8, 16).T.astype(np.int16)
        idx16 = np.tile(i16, (8, 1))
        per_core.append((np.ascontiguousarray(idx16),
                         np.ascontiguousarray(drel)))
    return per_core, tq, regs, perms


def _build_program(tq, regs):
    nc = bacc.Bacc(None, target_bir_lowering=False, num_swdge_queues=4)
    dt = mybir.dt
    P = nc.declare_dram_parameter
    TOT4 = int(tq.sum())
    xT = P("xT", [IN_CH, NS], dt.float32, isOutput=False)
    w1p = P("w1p", [128, 128], dt.float32, isOutput=False)
    w2 = P("w2", [HID, HID], dt.float32, isOutput=False)
    wh1 = P("wh1", [HID, HID2], dt.float32, isOutput=False)
    wh2 = P("wh2", [HID2, OUT], dt.float32, isOutput=False)
    b1f = P("b1f", [128, HID], dt.float32, isOutput=False)
    b2f = P("b2f", [128, HID], dt.float32, isOutput=False)
    bh1 = P("bh1", [HID2, 1], dt.float32, isOutput=False)
    bh2 = P("bh2", [OUT, 1], dt.float32, isOutput=False)
    dinvP = P("dinvP", [128, NTILE], dt.float32, isOutput=False)
    idxD = P("idxD", [128, TOT4 * 8], dt.int16, isOutput=False)
    dstrelP = P("dstrelP", [128, TOT4], dt.float32, isOutput=False)
    iotaP = P("iotaP", [128, 1024], dt.float32, isOutput=False)
    outT = P("outT", [OUT, NS], dt.float32, isOutput=True)

    # table rows padded to 128 bf16 cols (= 256 B, dma_gather min elem);
    # cols 64:128 are never written nor read (junk travels, is ignored).
    # 4 sub-slices / sub-tables per layer so each sub-AllGather fires as
    # soon as its 25 producer tiles are written (pipelined exchange).
    slice_d = [[nc.dram_tensor(f"slice{l}_{k}", [SUB, 128], dt.bfloat16)
                for k in range(NQ)] for l in (1, 2)]
    table_d = [[nc.dram_tensor(f"table{l}_{k}", [QROWS, 128], dt.bfloat16)
                for k in range(NQ)] for l in (1, 2)]

    iseq = mybir.AluOpType.is_equal
    relu = mybir.ActivationFunctionType.Relu
    copyf = mybir.ActivationFunctionType.Copy

    # per-tile (quarter, block) schedule; quarter-major then block
    col_block, col_q = [], []
    for q in range(NQ):
        for b in range(NBLK):
            col_block += [b] * int(tq[q, b])
            col_q += [q] * int(tq[q, b])
    # segment boundaries: tile index ranges per (q, b)
    seg_start = {}
    seg_stop = {}
    pos = 0
    qoff = [0]
    for q in range(NQ):
        for b in range(NBLK):
            t = int(tq[q, b])
            seg_start[pos] = (q, b)
            seg_stop[pos + t - 1] = (q, b)
            pos += t
        qoff.append(pos)

    with TileContext(nc) as tc:
        with tc.tile_pool(name="const", bufs=1) as cp, \
             tc.tile_pool(name="acc", bufs=1) as ap_, \
             tc.tile_pool(name="gath", bufs=10) as gp, \
             tc.tile_pool(name="work", bufs=3) as wp, \
             tc.tile_pool(name="m8", bufs=10) as mp, \
             tc.tile_pool(name="pst", bufs=2, space="PSUM") as ptp, \
             tc.tile_pool(name="psa", bufs=4, space="PSUM") as pap, \
             tc.tile_pool(name="psg", bufs=2, space="PSUM") as pgp:
            w1sb = cp.tile([128, 128], dt.float32)
            nc.sync.dma_start(out=w1sb[:], in_=w1p[:])
            w2sb = cp.tile([HID, HID], dt.float32)
            nc.sync.dma_start(out=w2sb[:], in_=w2[:])
            wh1sb = cp.tile([HID, HID2], dt.float32)
            nc.sync.dma_start(out=wh1sb[:], in_=wh1[:])
            wh2sb = cp.tile([HID2, OUT], dt.float32)
            nc.sync.dma_start(out=wh2sb[:], in_=wh2[:])
            b1sb = cp.tile([128, HID], dt.float32)
            nc.sync.dma_start(out=b1sb[:], in_=b1f[:])
            b2sb = cp.tile([128, HID], dt.float32)
            nc.sync.dma_start(out=b2sb[:], in_=b2f[:])
            bh1sb = cp.tile([HID2, 1], dt.float32)
            nc.sync.dma_start(out=bh1sb[:], in_=bh1[:])
            bh2sb = cp.tile([OUT, 1], dt.float32)
            nc.sync.dma_start(out=bh2sb[:], in_=bh2[:])
            dsb = cp.tile([128, NTILE], dt.float32)
            nc.sync.dma_start(out=dsb[:], in_=dinvP[:])
            ident = cp.tile([128, 128], dt.float32)
            make_identity(nc, ident[:])
            iotaf = cp.tile([128, 1024], dt.float32)
            nc.sync.dma_start(out=iotaf[:], in_=iotaP[:])
            iotab = cp.tile([128, 1024], dt.bfloat16)
            nc.vector.tensor_copy(iotab[:], iotaf[:])
            drf = cp.tile([128, TOT4], dt.float32)
            nc.sync.dma_start(out=drf[:], in_=dstrelP[:])
            drb = cp.tile([128, TOT4], dt.bfloat16)
            nc.vector.tensor_copy(drb[:], drf[:])
            idxsb = cp.tile([128, TOT4 * 8], dt.int16)
            nc.sync.dma_start(out=idxsb[:], in_=idxD[:])

            acc = ap_.tile([128, NBLK * HID], dt.float32)
            hwself = ap_.tile([128, NTILE * HID], dt.float32)

            def allgather(layer, k):
                nc.gpsimd.collective_compute(
                    "AllGather", mybir.AluOpType.bypass,
                    replica_groups=[list(range(8))],
                    ins=[slice_d[layer][k][:]],
                    outs=[table_d[layer][k][:]])

            def gemm1():
                for m in range(NTILE):
                    mc = slice(m * 128, (m + 1) * 128)
                    xa = wp.tile([128, 128], dt.float32, tag="xa")
                    nc.sync.dma_start(out=xa[:], in_=xT[0:128, mc])
                    xb = wp.tile([128, 128], dt.float32, tag="xb")
                    nc.sync.dma_start(out=xb[:], in_=xT[128:256, mc])
                    ps = pgp.tile([128, HID], dt.float32, tag="ps")
                    nc.tensor.matmul(ps[:], xa[:], w1sb[:, 0:HID],
                                     start=True, stop=False)
                    nc.tensor.matmul(ps[:], xb[:], w1sb[:, HID:128],
                                     start=False, stop=True)
                    hsl = hwself[:, m * HID:(m + 1) * HID]
                    nc.vector.tensor_scalar_mul(hsl, ps[:], dsb[:, m:m + 1])
                    sb16 = wp.tile([128, HID], dt.bfloat16, tag="sb16")
                    nc.scalar.activation(sb16[:], hsl, copyf)
                    nc.sync.dma_start(
                        out=slice_d[0][m // 25][
                            (m % 25) * 128:(m % 25 + 1) * 128, 0:HID],
                        in_=sb16[:])
                    if m % 25 == 24:
                        allgather(0, m // 25)

            def prime_gpool():
                for _ in range(10):
                    gt = gp.tile([128, CALL_T * 128], dt.bfloat16, tag="g")
                    nc.gpsimd.memset(gt[:], 0.0)

            def aggregate(layer, on_block_done):
                cur = None
                ncall = 0
                ci = 0
                pos0 = 0
                for q in range(NQ):
                    for b_ in range(NBLK):
                        T = int(tq[q, b_])
                        for t0 in range(0, T, CALL_T):
                            nw = min(CALL_T, T - t0)
                            c0 = pos0 + t0
                            reg = regs[ci]
                            ci += 1
                            # tiles fully inside the skip region have
                            # all-PAD masks (exactly zero contribution):
                            # drop their gather slots and matmuls, and move
                            # the segment stop to the last processed tile.
                            nproc = min(nw, -(-reg // 128))
                            last_call = t0 + nw == T
                            g = gp.tile([128, CALL_T * 128], dt.bfloat16,
                                        tag="g")
                            nc.gpsimd.dma_gather(
                                out_ap=g[:, 0:nproc * 128].rearrange(
                                    "p (k o) -> p k o", o=128),
                                in_ap=table_d[layer][q][:],
                                idxs_ap=idxsb[:, c0 * 8:(c0 + nproc) * 8],
                                num_idxs=nproc * 128,
                                num_idxs_reg=reg,
                                elem_size=128,
                                queue_num=ncall % 4,
                            )
                            ncall += 1
                            for c in range(c0, c0 + nproc):
                                k = c - c0
                                if k % 8 == 0:
                                    m8 = mp.tile([128, 1024], dt.bfloat16,
                                                 tag="m8")
                                    nm = min(8, nproc - k)
                                    d3b = drb[:, c:c + nm] \
                                        .rearrange("p (k o) -> p k o", o=1) \
                                        .broadcast_to((128, nm, 128))
                                    nc.vector.tensor_tensor(
                                        m8[:, 0:nm * 128].rearrange(
                                            "p (k o) -> p k o", o=128),
                                        iotab[:, 0:nm * 128].rearrange(
                                            "p (k o) -> p k o", o=128),
                                        d3b, iseq)
                                b = col_block[c]
                                if c in seg_start:
                                    agg_t = pap.tile([128, HID],
                                                     dt.float32, tag="agg")
                                    cur = agg_t
                                stop = last_call and c == c0 + nproc - 1
                                nc.tensor.matmul(
                                    cur[:],
                                    m8[:, (k % 8) * 128:(k % 8 + 1) * 128],
                                    g[:, k * 128:k * 128 + HID],
                                    start=c in seg_start,
                                    stop=stop)
                                if stop:
                                    asl = acc[:, b * HID:(b + 1) * HID]
                                    if q == 0:
                                        nc.vector.tensor_copy(asl, cur[:])
                                    else:
                                        nc.vector.tensor_add(asl, asl,
                                                             cur[:])
                                    if q == NQ - 1:
                                        on_block_done(b)
                        pos0 += T

            def pointwise_tile(layer, m):
                mc = slice(m * 128, (m + 1) * 128)
                hsl = hwself[:, m * HID:(m + 1) * HID]
                s = wp.tile([128, HID], dt.float32, tag="s")
                if m < NBLK:
                    nc.vector.tensor_add(
                        s[:], acc[:, m * HID:(m + 1) * HID], hsl)
                else:
                    nc.vector.tensor_copy(s[:], hsl)
                nc.vector.tensor_scalar_mul(s[:], s[:], dsb[:, m:m + 1])
                nc.vector.tensor_add(s[:], s[:],
                                     b1sb[:] if layer == 0 else b2sb[:])
                h = wp.tile([128, HID], dt.float32, tag="h")
                nc.scalar.activation(h[:], s[:], relu)
                pt = ptp.tile([128, 128], dt.float32, tag="tp")
                nc.tensor.transpose(pt[0:HID, :], h[:], ident[:])
                ht = wp.tile([HID, 128], dt.float32, tag="ht")
                nc.scalar.activation(ht[:], pt[0:HID, :], copyf)
                if layer == 0:
                    ps2 = pgp.tile([128, HID], dt.float32, tag="ps")
                    nc.tensor.matmul(ps2[:], ht[:], w2sb[:],
                                     start=True, stop=True)
                    nc.vector.tensor_scalar_mul(hsl, ps2[:],
                                                dsb[:, m:m + 1])
                    sb16 = wp.tile([128, HID], dt.bfloat16, tag="sb16b")
                    nc.scalar.activation(sb16[:], hsl, copyf)
                    nc.sync.dma_start(
                        out=slice_d[1][m // 25][
                            (m % 25) * 128:(m % 25 + 1) * 128, 0:HID],
                        in_=sb16[:])
                    if m % 25 == 24:
                        allgather(1, m // 25)
                else:
                    pz = ptp.tile([128, 128], dt.float32, tag="tp")
                    nc.tensor.matmul(pz[0:HID2, :], wh1sb[:], ht[:],
                                     start=True, stop=True)
                    zb = wp.tile([HID2, 128], dt.float32, tag="zb")
                    nc.scalar.activation(zb[:], pz[0:HID2, :], relu,
                                         bias=bh1sb[:])
                    po = ptp.tile([128, 128], dt.float32, tag="tp")
                    nc.tensor.matmul(po[0:OUT, :], wh2sb[:], zb[:],
                                     start=True, stop=True)
                    ob = wp.tile([OUT, 128], dt.float32, tag="ob")
                    nc.vector.tensor_scalar_add(ob[:], po[0:OUT, :],
                                                bh2sb[:])
                    nc.sync.dma_start(out=outT[:, mc], in_=ob[:])

            def on_block_done(layer):
                # pointwise for block b right after its final fold; tiles
                # 98/99 (self-only) ride along with block 97 so the last
                # sub-slice / output completes without a separate pass.
                def f(b):
                    pointwise_tile(layer, b)
                    if b == NBLK - 1:
                        for m in range(NBLK, NTILE):
                            pointwise_tile(layer, m)
                return f

            prime_gpool()
            gemm1()
            aggregate(0, on_block_done(0))
            aggregate(1, on_block_done(1))

    nc.finalize()
    return nc


def kernel(x, edge_index, W1, b1, W2, b2, Wh1, bh1, Wh2, bh2, _trace=False):
    x = np.asarray(x, np.float32)
    src = np.asarray(edge_index[0], np.int64)
    dst = np.asarray(edge_index[1], np.int64)

    per_core, tq, regs, perms = _build_schedule(src, dst)
    sig = tuple(tq.ravel()) + tuple(regs)
    if sig not in _compiled:
        _compiled[sig] = _build_program(tq, regs)
    nc = _compiled[sig]

    deg = np.bincount(dst, minlength=N).astype(np.float64) + 1.0
    dinv = (1.0 / np.sqrt(deg)).astype(np.float32)

    W1 = np.asarray(W1, np.float32)
    w1p = np.concatenate([W1[:128], W1[128:]], axis=1)
    b1f = np.tile(np.asarray(b1, np.float32)[None, :], (128, 1))
    b2f = np.tile(np.asarray(b2, np.float32)[None, :], (128, 1))
    bh1c = np.asarray(bh1, np.float32)[:, None]
    bh2c = np.asarray(bh2, np.float32)[:, None]
    iota = np.tile(np.arange(128, dtype=np.float32)[None, :], (128, 8))

    in_maps = []
    for c in range(8):
        idx16, drel = per_core[c]
        xs = np.zeros((NS, IN_CH), np.float32)
        xs[perms[c]] = x[c * NS_RAW:(c + 1) * NS_RAW]
        dv = np.ones(NS, np.float32)
        dv[perms[c]] = dinv[c * NS_RAW:(c + 1) * NS_RAW]
        in_maps.append({
            "xT": np.ascontiguousarray(xs.T),
            "w1p": np.ascontiguousarray(w1p),
            "w2": np.asarray(W2, np.float32),
            "wh1": np.asarray(Wh1, np.float32),
            "wh2": np.asarray(Wh2, np.float32),
            "b1f": b1f, "b2f": b2f, "bh1": bh1c, "bh2": bh2c,
            "dinvP": np.ascontiguousarray(dv.reshape(NTILE, 128).T),
            "idxD": idx16,
            "dstrelP": drel,
            "iotaP": iota,
        })

    res = run_bass_kernel_spmd(nc, in_maps, list(range(8)), trace=_trace)
    out = np.empty((N, OUT), np.float32)
    for c in range(8):
        out[c * NS_RAW:(c + 1) * NS_RAW] = res.results[c]["outT"].T[perms[c]]
    if _trace:
        kernel.last_results = res
    return out
